# revision 60
# baseline (speedup 1.0000x reference)
"""Distributed Trainium2 kernel: Gemma-style attention block (B=2,T=2048,H=2048,
NH=16,NKV=4,HD=128) across 8 NeuronCores.

Sharding: batch x head-group. Core c handles batch c//4 with q heads
{4g..4g+3} (g = c%4) and kv head g (GQA groups align exactly).  Activations
are kept feature-major ([d_part, t_free]) so every matmul contracts on the
partition dim.  Softmax is max-free (safe: rmsnorm bounds |scores| <=
sqrt(HD)); denominators and rmsnorm sum-of-squares are computed pre-broadcast
via an all-ones stationary matmul.  The 4 per-batch o_proj partials are summed
on host.

Perf structure (hill-climbed against NTFF traces):
- x pre-tiled host-side so each 512-token block is ONE contiguous 2MB DMA
- constants split: qkv weights (first-MM gate) in sub-DMAs on the ACT ring,
  rope tables next, o_proj weights + causal microblock mask last
- phase1 rmsnorm: Square on ScalarE straight from PSUM; rstd =
  exp(-0.5*ln(ssq/HD)) on ScalarE (DVE reciprocal is 6 cpe - too slow)
- phase2 causal triangle: S^T/exp/den/PV restricted to valid query columns;
  single shared [128,128] upper-tri mask on the diagonal microblock only;
  S^T matmuls software-pipelined DEPTH tiles ahead of the ACT exp
- phase3 o_proj: 2048-wide output rows, PSUM->SBUF copies 3:1 VectorE/ScalarE,
  one 512KB output DMA per 128-token row
"""

import os
import sys

sys.path.insert(0, "/opt/trn_rl_repo")

import numpy as np
import ml_dtypes

import concourse.bass as bass
import concourse.mybir as mybir
import concourse.tile as tile
from concourse.bass_utils import run_bass_kernel_spmd

BF16 = ml_dtypes.bfloat16

B, T, H = 2, 2048, 2048
NH, NKV, HD = 16, 4, 128
THETA = 10000.0
NCORES = 8
GPB = 4                    # head-groups (cores) per batch
QHL = NH // GPB // B * 2   # 4 q heads per core
BT = B * T
NBLK = T // 512            # 4 blocks of 512 tokens per batch
NFT = QHL + 2              # feature tiles per ht: q0..q3, k, v
SCALE = 1.0 / np.sqrt(HD)

LAST_RESULTS = None        # stash for test harness profiling

# packed constants W [128, NCW]: qkv weights (first-MM gate)
NCW = 16 * NFT * 128       # per ht: 6 x 128 feature cols
# packed constants A [128, NCA]: rope tables
OFF_CQ = 0                 # 2048
OFF_CK = OFF_CQ + T        # 2048
OFF_SIN = OFF_CK + T       # 2048
OFF_RQ = OFF_SIN + T       # 128
OFF_RK = OFF_RQ + HD       # 128
OFF_ONES = OFF_RK + HD     # 128
NCA = OFF_ONES + 128
# packed constants B [128, NCB]: o_proj weights + causal microblock mask
OFF_WO = 0                 # QHL*2048
OFF_TRI = OFF_WO + QHL * H
NCB = OFF_TRI + 128


def _rope_tables(w_q, w_k):
    """rope(w*q) = cosw * q + sin * (R_w @ q) where cosw = cos*(1+w) and
    R_w = rot_half matrix with the +-1 and the (1+w) source weight folded in.
    Returns cosw_q, cosw_k, sin (plain), rotmT_q, rotmT_k (lhsT layout)."""
    inv = 1.0 / (THETA ** (np.arange(0, HD, 2, dtype=np.float64) / HD))  # [64]
    t = np.arange(T, dtype=np.float64)
    fr = np.outer(inv, t)                      # [64, T]
    emb = np.concatenate([fr, fr], 0)          # [HD, T]
    cos, sin = np.cos(emb), np.sin(emb)
    cosws, rotms = [], []
    for w in (w_q, w_k):
        wp = 1.0 + w.astype(np.float64)
        cosws.append((cos * wp[:, None]).astype(BF16))
        R = np.zeros((HD, HD))
        for m in range(64):
            R[m, m + 64] = -wp[m + 64]
        for m in range(64, HD):
            R[m, m - 64] = +wp[m - 64]
        rotms.append(np.ascontiguousarray(R.T).astype(BF16))  # lhsT[k, m] = R[m, k]
    return cosws[0], cosws[1], sin.astype(BF16), rotms[0], rotms[1]


def _legalize_waits(nc):
    """This container's walrus accepts only ONE sync wait per instruction
    (even shipped Tile kernels fail codegen). Split each multi-wait
    instruction into single-wait NOPs on the same engine followed by the
    original holding the last wait — per-engine program order makes this
    exactly equivalent."""
    nid = 0
    for fn in nc.m.functions:
        for blk in fn.blocks:
            out = []
            for inst in blk.instructions:
                si = getattr(inst, "sync_info", None)
                if si is not None and si.on_wait and len(si.on_wait) > 1:
                    waits = list(si.on_wait)
                    ups = list(si.on_update) if si.on_update else []
                    for w in waits[:-1]:
                        nop = mybir.InstNoOp(name=f"swx-{nid}", ins=[], outs=[])
                        nid += 1
                        nop.engine = inst.engine
                        nop.sync_info = mybir.SyncInfo(on_wait=[w], on_update=[])
                        out.append(nop)
                    inst.sync_info = mybir.SyncInfo(
                        on_wait=[waits[-1]], on_update=ups)
                out.append(inst)
            blk.instructions = out
    return nc


def _build_graph(cfg=None):
    cfg = {**dict(xtp=3, tmp=4, pacc=2, pden=2, pmm=4, depth=3), **(cfg or {})}
    nc = bass.Bass()
    f32, bf16 = mybir.dt.float32, mybir.dt.bfloat16

    # x pre-tiled on host (this core's batch): row bi*128+p, col ht*512+c
    xB = nc.dram_tensor("xB", [NBLK * 128, 16 * 512], bf16, kind="ExternalInput")
    constsW = nc.dram_tensor("constsW", [128, NCW], bf16, kind="ExternalInput")
    constsA = nc.dram_tensor("constsA", [128, NCA], bf16, kind="ExternalInput")
    constsB = nc.dram_tensor("constsB", [128, NCB], bf16, kind="ExternalInput")
    out = nc.dram_tensor("out", [T, H], bf16, kind="ExternalOutput")

    with tile.TileContext(nc) as tc:
        with (
            tc.tile_pool(name="singles", bufs=1) as singles,
            tc.tile_pool(name="xtp", bufs=cfg["xtp"]) as xtp,
            tc.tile_pool(name="tmp", bufs=cfg["tmp"]) as tmp,
            tc.tile_pool(name="psum", bufs=cfg["pacc"], space="PSUM") as pacc,
            tc.tile_pool(name="psden", bufs=cfg["pden"], space="PSUM") as pden,
            tc.tile_pool(name="psmm", bufs=cfg["pmm"], space="PSUM") as pmm,
        ):
            # ---- resident constants ----
            constsW_sb = singles.tile([128, NCW], bf16)
            constsA_sb = singles.tile([128, NCA], bf16)
            constsB_sb = singles.tile([128, NCB], bf16)
            for q6 in range(6):     # sub-DMAs: first accum MMs start sooner
                c0, c1 = q6 * 2048, min((q6 + 1) * 2048, NCW)
                nc.scalar.dma_start(
                    out=constsW_sb[:, c0:c1], in_=constsW[:, c0:c1])
                if q6 == 0:
                    # rope tables are needed ~3us after the first accum
                    # chain; don't let them queue behind x prefetches
                    nc.scalar.dma_start(out=constsA_sb, in_=constsA[:, :])
            wqkv_sb = constsW_sb
            cq_sb = constsA_sb[:, OFF_CQ:OFF_CQ + T]
            ck_sb = constsA_sb[:, OFF_CK:OFF_CK + T]
            sin_sb = constsA_sb[:, OFF_SIN:OFF_SIN + T]
            rq_sb = constsA_sb[:, OFF_RQ:OFF_RQ + HD]
            rk_sb = constsA_sb[:, OFF_RK:OFF_RK + HD]
            ones_sb = constsA_sb[:, OFF_ONES:OFF_ONES + 128]
            wo_sb = constsB_sb[:, OFF_WO:OFF_WO + QHL * H]
            tri_sb = constsB_sb[:, OFF_TRI:OFF_TRI + 128]

            # ---- activations (feature-major), one batch per core ----
            qT = singles.tile([128, QHL * T], bf16, name="qT", tag="qT")
            kT = singles.tile([128, T], bf16, name="kT", tag="kT")
            vn = singles.tile([128, 16 * 128], bf16, name="vn", tag="vn")
            attnT = singles.tile([128, QHL * T], bf16, name="attnT", tag="attnT")

            ACT_F = mybir.ActivationFunctionType
            FT = NFT * 128          # 768 feature cols per ht in wqkv

            def phase1_block(blk):
                t0 = blk * 512
                xt_all = xtp.tile([128, 16 * 512], bf16, tag="xt")
                if blk == 0:        # cold start: split so ht=0 MMs start early
                    for q4 in range(4):
                        nc.sync.dma_start(
                            out=xt_all[:, q4 * 2048:(q4 + 1) * 2048],
                            in_=xB[0:128, q4 * 2048:(q4 + 1) * 2048])
                elif blk >= 2:      # late blocks slot-release behind o_proj
                    # output DMAs: halves let the accum chain start early
                    for q2 in range(2):
                        nc.sync.dma_start(
                            out=xt_all[:, q2 * 4096:(q2 + 1) * 4096],
                            in_=xB[blk * 128:(blk + 1) * 128,
                                   q2 * 4096:(q2 + 1) * 4096])
                else:
                    nc.sync.dma_start(
                        out=xt_all, in_=xB[blk * 128:(blk + 1) * 128, :])
                xts = [xt_all[:, ht * 512:(ht + 1) * 512] for ht in range(16)]
                # k first (diagonal S tiles of the next attention group gate
                # on it), then q0..q3 (feature-major out)
                for dt in [QHL] + list(range(QHL)):
                    ps = pacc.tile([128, 512], f32, tag="acc")
                    for ht in range(16):
                        nc.tensor.matmul(
                            ps,
                            lhsT=wqkv_sb[:, dt * 2048 + ht * 128:dt * 2048 + (ht + 1) * 128],
                            rhs=xts[ht], start=(ht == 0), stop=(ht == 15))
                    sq = tmp.tile([128, 512], bf16, tag="sq")
                    nc.scalar.activation(out=sq, in_=ps, func=ACT_F.Square)
                    traw = tmp.tile([128, 512], bf16, tag="traw")
                    nc.scalar.activation(out=traw, in_=ps, func=ACT_F.Copy)
                    ssq = pmm.tile([128, 512], f32, tag="mm")
                    nc.tensor.matmul(ssq, lhsT=ones_sb, rhs=sq, start=True, stop=True)
                    # rstd = exp(-0.5*ln(ssq/HD)) = 1/sqrt(ssq/HD) on ScalarE
                    lssq = tmp.tile([128, 512], f32, tag="std")
                    nc.scalar.activation(
                        out=lssq, in_=ssq, func=ACT_F.Ln, scale=1.0 / HD)
                    rstd = tmp.tile([128, 512], bf16, tag="rstd")
                    nc.scalar.activation(
                        out=rstd, in_=lssq, func=ACT_F.Exp, scale=-0.5)
                    cos_t, rot_t = (cq_sb, rq_sb) if dt < QHL else (ck_sb, rk_sb)
                    t1 = tmp.tile([128, 512], bf16, tag="t1")
                    nc.vector.tensor_mul(t1, traw, cos_t[:, t0:t0 + 512])
                    rps = pmm.tile([128, 512], f32, tag="mm")
                    nc.tensor.matmul(rps, lhsT=rot_t, rhs=traw, start=True, stop=True)
                    t2 = tmp.tile([128, 512], bf16, tag="t2")
                    nc.vector.tensor_mul(t2, rps, sin_sb[:, t0:t0 + 512])
                    nc.vector.tensor_add(out=t1, in0=t1, in1=t2)
                    dest = (qT[:, dt * T + t0:dt * T + t0 + 512] if dt < QHL
                            else kT[:, t0:t0 + 512])
                    nc.vector.tensor_mul(dest, t1, rstd)
                # v projection, natural layout [t_part, d_free]
                vps = pacc.tile([128, 512], f32, tag="acc")
                for c4 in range(4):
                    for ht in range(16):
                        nc.tensor.matmul(
                            vps[:, c4 * 128:(c4 + 1) * 128],
                            lhsT=xts[ht][:, c4 * 128:(c4 + 1) * 128],
                            rhs=wqkv_sb[:, 5 * 2048 + ht * 128:5 * 2048 + (ht + 1) * 128],
                            start=(ht == 0), stop=(ht == 15))
                with nc.allow_low_precision(reason="bf16 act copy"):
                    nc.vector.tensor_copy(
                        out=vn[:, blk * 512:(blk + 1) * 512], in_=vps)

            def attn_block(h, j):
                # Software-pipelined: S^T matmuls issued DEPTH tiles ahead so
                # the PE never stalls on the ACT exp of the current tile.
                # Causal restriction: tile i >= 4j only covers query columns
                # >= 128*(i-4j); the 128-wide boundary microblock gets the
                # shared upper-tri mask.
                DEPTH = cfg["depth"]
                ntk = 4 * j + 4
                aps = pacc.tile([128, 512], f32, tag="acc")
                dps = pden.tile([128, 512], f32, tag="den")
                sps_l, pt_l, c0_l = [], [], []

                def issue_st(i):
                    r = i - 4 * j
                    c0 = 128 * r if r > 0 else 0
                    sps = pmm.tile([128, 512], f32, tag="mm", name="sps")
                    nc.tensor.matmul(
                        sps[:, c0:], lhsT=kT[:, i * 128:(i + 1) * 128],
                        rhs=qT[:, h * T + j * 512 + c0:h * T + (j + 1) * 512],
                        start=True, stop=True)
                    sps_l.append(sps)
                    c0_l.append(c0)

                def issue_exp(i):
                    r = i - 4 * j
                    c0 = c0_l[i]
                    pt = tmp.tile([128, 512], bf16, tag="pt", name="pt")
                    nc.scalar.activation(
                        out=pt[:, c0:], in_=sps_l[i][:, c0:],
                        func=ACT_F.Exp, scale=SCALE)
                    if r >= 0:
                        # GpSimd (idle): keeps the microblock mask from
                        # queueing behind 3.3us DVE reciprocals
                        nc.gpsimd.tensor_mul(
                            pt[:, c0:c0 + 128], pt[:, c0:c0 + 128], tri_sb)
                    pt_l.append(pt)

                for i in range(min(DEPTH, ntk)):
                    issue_st(i)
                issue_exp(0)
                for i in range(ntk):
                    if i + DEPTH < ntk:
                        issue_st(i + DEPTH)
                    if i + 1 < ntk:
                        issue_exp(i + 1)
                    c0 = c0_l[i]
                    nc.tensor.matmul(dps[:, c0:], lhsT=ones_sb, rhs=pt_l[i][:, c0:],
                                     start=(i == 0), stop=(i == ntk - 1))
                    nc.tensor.matmul(aps[:, c0:], lhsT=vn[:, i * 128:(i + 1) * 128],
                                     rhs=pt_l[i][:, c0:], start=(i == 0),
                                     stop=(i == ntk - 1))
                recip = tmp.tile([128, 512], mybir.dt.float32, tag="rec")
                if h == QHL - 1:
                    # last head gates the next o_proj group: use the shorter
                    # ACT Ln->Exp chain instead of the 3.3us DVE reciprocal
                    lr = tmp.tile([128, 512], mybir.dt.float32, tag="lrec")
                    nc.scalar.activation(out=lr, in_=dps, func=ACT_F.Ln)
                    nc.scalar.activation(out=recip, in_=lr, func=ACT_F.Exp,
                                         scale=-1.0)
                else:
                    nc.vector.reciprocal(out=recip, in_=dps)
                nc.vector.tensor_mul(
                    attnT[:, h * T + j * 512:h * T + (j + 1) * 512], aps, recip)

            def oproj_row(m):
                # full 2048-wide output row of 128 tokens: 4 accumulations into
                # separate PSUM tiles, copies gathered into one SBUF tile, ONE
                # output DMA (512KB, 4KB/partition contiguous).
                osb = tmp.tile([128, 2048], bf16, tag="osb", name="osb")
                for j in range(NBLK):
                    ops = pmm.tile([128, 512], f32, tag="mm", name="ops")
                    for hh in range(QHL):
                        nc.tensor.matmul(
                            ops,
                            lhsT=attnT[:, hh * T + m * 128:hh * T + (m + 1) * 128],
                            rhs=wo_sb[:, hh * H + j * 512:hh * H + (j + 1) * 512],
                            start=(hh == 0), stop=(hh == QHL - 1))
                    if m >= 8 or (m + j) % 4 < 3:
                        with nc.allow_low_precision(reason="bf16 partials, host-summed f32"):
                            nc.vector.tensor_copy(
                                out=osb[:, j * 512:(j + 1) * 512], in_=ops)
                    else:
                        nc.scalar.activation(
                            out=osb[:, j * 512:(j + 1) * 512], in_=ops,
                            func=ACT_F.Copy)
                nc.sync.dma_start(
                    out=out[m * 128:(m + 1) * 128, :], in_=osb)

            # phase interleave: attention for query block j only needs
            # k/v/q blocks <= j; o_proj rows 4j..4j+3 only need attnT of
            # block j (all heads). Spreads ACT/DVE/DMA phases into the
            # PE-dense projection phase.
            phase1_block(0)
            # o_proj weights + mask load deferred past the cold-start
            # transfers, but early enough to beat the first o_proj row
            nc.scalar.dma_start(out=constsB_sb, in_=constsB[:, :])
            phase1_block(1)
            for h in range(QHL):
                attn_block(h, 0)
            phase1_block(2)
            for m in range(0, 4):
                oproj_row(m)
            for h in range(QHL):
                attn_block(h, 1)
            phase1_block(3)
            for m in range(4, 8):
                oproj_row(m)
            for h in range(QHL):
                attn_block(h, 2)
            for m in range(8, 12):
                oproj_row(m)
            for h in range(QHL):
                attn_block(h, 3)
            for m in range(12, 16):
                oproj_row(m)
    return nc


_GRAPH = None


def kernel(x, Wq, Wk, Wv, Wo, q_norm_w, k_norm_w):
    global _GRAPH, LAST_RESULTS
    x = np.asarray(x, dtype=np.float32)
    Wq = np.asarray(Wq, dtype=np.float32)
    Wk = np.asarray(Wk, dtype=np.float32)
    Wv = np.asarray(Wv, dtype=np.float32)
    Wo = np.asarray(Wo, dtype=np.float32)
    q_norm_w = np.asarray(q_norm_w, dtype=np.float32)
    k_norm_w = np.asarray(k_norm_w, dtype=np.float32)

    xT = np.ascontiguousarray(x.reshape(BT, H).T).astype(BF16)
    # pre-tiled blocks: xBm[bi*128+p, ht*512+c] = xT[ht*128+p, bi*512+c]
    xBm = np.ascontiguousarray(
        xT.reshape(16, 128, B * NBLK, 512).transpose(2, 1, 0, 3)
        .reshape(B * NBLK * 128, 16 * 512))
    cos_q, cos_k, sin_d, rotm_q, rotm_k = _rope_tables(q_norm_w, k_norm_w)
    p = np.arange(128)[:, None]
    f = np.arange(128)[None, :]
    tri = (f >= p).astype(BF16)       # upper-tri incl diagonal

    in_maps = []
    for c in range(NCORES):
        b, g = c // GPB, c % GPB
        w_all = np.concatenate([
            Wq[QHL * HD * g:QHL * HD * (g + 1)],
            Wk[HD * g:HD * (g + 1)],
            Wv[HD * g:HD * (g + 1)]], 0)              # [768, H]
        wqkvT = np.ascontiguousarray(w_all.T).astype(BF16)       # [H, 768]
        woT = np.ascontiguousarray(
            Wo[:, QHL * HD * g:QHL * HD * (g + 1)].T).astype(BF16)  # [QHL*HD, H]
        # dt-major packing: col = dt*2048 + ht*128 + q, so each 512KB
        # weight sub-DMA delivers one full feature tile in consumption order
        cw = np.ascontiguousarray(
            wqkvT.reshape(16, 128, NFT, 128).transpose(1, 2, 0, 3)
            .reshape(128, NCW)).astype(BF16)
        ca = np.zeros((128, NCA), dtype=BF16)
        ca[:, OFF_CQ:OFF_CQ + T] = cos_q
        ca[:, OFF_CK:OFF_CK + T] = cos_k
        ca[:, OFF_SIN:OFF_SIN + T] = sin_d
        ca[:, OFF_RQ:OFF_RQ + HD] = rotm_q
        ca[:, OFF_RK:OFF_RK + HD] = rotm_k
        ca[:, OFF_ONES:OFF_ONES + 128] = 1.0
        cb = np.zeros((128, NCB), dtype=BF16)
        cb[:, OFF_WO:OFF_WO + QHL * H] = (
            woT.reshape(QHL, 128, H).transpose(1, 0, 2).reshape(128, QHL * H))
        cb[:, OFF_TRI:OFF_TRI + 128] = tri
        in_maps.append({
            "xB": xBm[b * NBLK * 128:(b + 1) * NBLK * 128],
            "constsW": cw, "constsA": ca, "constsB": cb})

    if _GRAPH is None:
        import json as _json
        cfg = _json.loads(os.environ.get("ATTN_CFG", "{}")) or None
        _GRAPH = _legalize_waits(_build_graph(cfg=cfg))

    want_trace = bool(int(os.environ.get("ATTN_TRACE", "0")))
    try:
        res = run_bass_kernel_spmd(
            _GRAPH, in_maps, core_ids=list(range(NCORES)), trace=want_trace)
    except ModuleNotFoundError:
        if not want_trace:
            raise
        # axon NTFF profile hook unavailable in this environment
        res = run_bass_kernel_spmd(
            _GRAPH, in_maps, core_ids=list(range(NCORES)), trace=False)
    LAST_RESULTS = res
    acc = np.zeros((BT, H), dtype=np.float32)
    for c, r in enumerate(res.results):
        b = c // GPB
        acc[b * T:(b + 1) * T] += r["out"]
    return acc.reshape(B, T, H)


# revision 61
# speedup vs baseline: 1.0408x; 1.0408x over previous
"""Distributed Trainium2 kernel: Gemma-style attention block (B=2,T=2048,H=2048,
NH=16,NKV=4,HD=128) across 8 NeuronCores.

Sharding: batch x head-group. Core c handles batch c//4 with q heads
{4g..4g+3} (g = c%4) and kv head g (GQA groups align exactly).  Activations
are kept feature-major ([d_part, t_free]) so every matmul contracts on the
partition dim.  Softmax is max-free (safe: rmsnorm bounds |scores| <=
sqrt(HD)); denominators and rmsnorm sum-of-squares are computed pre-broadcast
via an all-ones stationary matmul.  The 4 per-batch o_proj partials are summed
on host.

Perf structure (hill-climbed against NTFF traces):
- x pre-tiled host-side so each 512-token block is ONE contiguous 2MB DMA
- constants split: qkv weights (first-MM gate) in sub-DMAs on the ACT ring,
  rope tables next, o_proj weights + causal microblock mask last
- phase1 rmsnorm: Square on ScalarE straight from PSUM; rstd =
  exp(-0.5*ln(ssq/HD)) on ScalarE (DVE reciprocal is 6 cpe - too slow)
- phase2 causal triangle: S^T/exp/den/PV restricted to valid query columns;
  single shared [128,128] upper-tri mask on the diagonal microblock only;
  S^T matmuls software-pipelined DEPTH tiles ahead of the ACT exp
- phase3 o_proj: 2048-wide output rows, PSUM->SBUF copies 3:1 VectorE/ScalarE,
  one 512KB output DMA per 128-token row
"""

import os
import sys

sys.path.insert(0, "/opt/trn_rl_repo")

import numpy as np
import ml_dtypes

import concourse.bass as bass
import concourse.mybir as mybir
import concourse.tile as tile
from concourse.bass_utils import run_bass_kernel_spmd

BF16 = ml_dtypes.bfloat16

B, T, H = 2, 2048, 2048
NH, NKV, HD = 16, 4, 128
THETA = 10000.0
NCORES = 8
GPB = 4                    # head-groups (cores) per batch
QHL = NH // GPB // B * 2   # 4 q heads per core
BT = B * T
NBLK = T // 512            # 4 blocks of 512 tokens per batch
NFT = QHL + 2              # feature tiles per ht: q0..q3, k, v
SCALE = 1.0 / np.sqrt(HD)

LAST_RESULTS = None        # stash for test harness profiling

# packed constants W [128, NCW]: qkv weights (first-MM gate)
NCW = 16 * NFT * 128       # per ht: 6 x 128 feature cols
# packed constants A [128, NCA]: rope tables
OFF_CQ = 0                 # 2048
OFF_CK = OFF_CQ + T        # 2048
OFF_SIN = OFF_CK + T       # 2048
OFF_RQ = OFF_SIN + T       # 128
OFF_RK = OFF_RQ + HD       # 128
OFF_ONES = OFF_RK + HD     # 128
NCA = OFF_ONES + 128
# packed constants B [128, NCB]: o_proj weights + causal microblock mask
OFF_WO = 0                 # QHL*2048
OFF_TRI = OFF_WO + QHL * H
NCB = OFF_TRI + 128


def _rope_tables(w_q, w_k):
    """rope(w*q) = cosw * q + sin * (R_w @ q) where cosw = cos*(1+w) and
    R_w = rot_half matrix with the +-1 and the (1+w) source weight folded in.
    Returns cosw_q, cosw_k, sin (plain), rotmT_q, rotmT_k (lhsT layout)."""
    inv = 1.0 / (THETA ** (np.arange(0, HD, 2, dtype=np.float64) / HD))  # [64]
    t = np.arange(T, dtype=np.float64)
    fr = np.outer(inv, t)                      # [64, T]
    emb = np.concatenate([fr, fr], 0)          # [HD, T]
    cos, sin = np.cos(emb), np.sin(emb)
    cosws, rotms = [], []
    for w in (w_q, w_k):
        wp = 1.0 + w.astype(np.float64)
        cosws.append((cos * wp[:, None]).astype(BF16))
        R = np.zeros((HD, HD))
        for m in range(64):
            R[m, m + 64] = -wp[m + 64]
        for m in range(64, HD):
            R[m, m - 64] = +wp[m - 64]
        rotms.append(np.ascontiguousarray(R.T).astype(BF16))  # lhsT[k, m] = R[m, k]
    return cosws[0], cosws[1], sin.astype(BF16), rotms[0], rotms[1]


def _legalize_waits(nc):
    """This container's walrus accepts only ONE sync wait per instruction
    (even shipped Tile kernels fail codegen). Split each multi-wait
    instruction into single-wait NOPs on the same engine followed by the
    original holding the last wait — per-engine program order makes this
    exactly equivalent."""
    nid = 0
    for fn in nc.m.functions:
        for blk in fn.blocks:
            out = []
            for inst in blk.instructions:
                si = getattr(inst, "sync_info", None)
                if si is not None and si.on_wait and len(si.on_wait) > 1:
                    waits = list(si.on_wait)
                    ups = list(si.on_update) if si.on_update else []
                    for w in waits[:-1]:
                        nop = mybir.InstNoOp(name=f"swx-{nid}", ins=[], outs=[])
                        nid += 1
                        nop.engine = inst.engine
                        nop.sync_info = mybir.SyncInfo(on_wait=[w], on_update=[])
                        out.append(nop)
                    inst.sync_info = mybir.SyncInfo(
                        on_wait=[waits[-1]], on_update=ups)
                out.append(inst)
            blk.instructions = out
    return nc


def _build_graph(cfg=None):
    cfg = {**dict(xtp=3, tmp=4, pacc=2, pden=2, pmm=4, depth=3), **(cfg or {})}
    nc = bass.Bass()
    f32, bf16 = mybir.dt.float32, mybir.dt.bfloat16

    # x pre-tiled on host (this core's batch): row bi*128+p, col ht*512+c
    xB = nc.dram_tensor("xB", [NBLK * 128, 16 * 512], bf16, kind="ExternalInput")
    constsW = nc.dram_tensor("constsW", [128, NCW], bf16, kind="ExternalInput")
    constsA = nc.dram_tensor("constsA", [128, NCA], bf16, kind="ExternalInput")
    constsB = nc.dram_tensor("constsB", [128, NCB], bf16, kind="ExternalInput")
    out = nc.dram_tensor("out", [T, H], bf16, kind="ExternalOutput")

    with tile.TileContext(nc) as tc:
        with (
            tc.tile_pool(name="singles", bufs=1) as singles,
            tc.tile_pool(name="xtp", bufs=cfg["xtp"]) as xtp,
            tc.tile_pool(name="tmp", bufs=cfg["tmp"]) as tmp,
            tc.tile_pool(name="psum", bufs=cfg["pacc"], space="PSUM") as pacc,
            tc.tile_pool(name="psden", bufs=cfg["pden"], space="PSUM") as pden,
            tc.tile_pool(name="psmm", bufs=cfg["pmm"], space="PSUM") as pmm,
        ):
            # ---- resident constants ----
            constsW_sb = singles.tile([128, NCW], bf16)
            constsA_sb = singles.tile([128, NCA], bf16)
            constsB_sb = singles.tile([128, NCB], bf16)
            for q6 in range(6):     # sub-DMAs: first accum MMs start sooner
                c0, c1 = q6 * 2048, min((q6 + 1) * 2048, NCW)
                nc.scalar.dma_start(
                    out=constsW_sb[:, c0:c1], in_=constsW[:, c0:c1])
                if q6 == 0:
                    # rope tables are needed ~3us after the first accum
                    # chain; don't let them queue behind x prefetches
                    nc.scalar.dma_start(out=constsA_sb, in_=constsA[:, :])
            wqkv_sb = constsW_sb
            cq_sb = constsA_sb[:, OFF_CQ:OFF_CQ + T]
            ck_sb = constsA_sb[:, OFF_CK:OFF_CK + T]
            sin_sb = constsA_sb[:, OFF_SIN:OFF_SIN + T]
            rq_sb = constsA_sb[:, OFF_RQ:OFF_RQ + HD]
            rk_sb = constsA_sb[:, OFF_RK:OFF_RK + HD]
            ones_sb = constsA_sb[:, OFF_ONES:OFF_ONES + 128]
            wo_sb = constsB_sb[:, OFF_WO:OFF_WO + QHL * H]
            tri_sb = constsB_sb[:, OFF_TRI:OFF_TRI + 128]

            # ---- activations (feature-major), one batch per core ----
            qT = singles.tile([128, QHL * T], bf16, name="qT", tag="qT")
            kT = singles.tile([128, T], bf16, name="kT", tag="kT")
            vn = singles.tile([128, 16 * 128], bf16, name="vn", tag="vn")
            attnT = singles.tile([128, QHL * T], bf16, name="attnT", tag="attnT")

            ACT_F = mybir.ActivationFunctionType
            FT = NFT * 128          # 768 feature cols per ht in wqkv

            def phase1_block(blk):
                t0 = blk * 512
                xt_all = xtp.tile([128, 16 * 512], bf16, tag="xt")
                if blk == 0:        # cold start: split so ht=0 MMs start early
                    for q4 in range(4):
                        nc.sync.dma_start(
                            out=xt_all[:, q4 * 2048:(q4 + 1) * 2048],
                            in_=xB[0:128, q4 * 2048:(q4 + 1) * 2048])
                elif blk >= 2:      # late blocks slot-release behind o_proj
                    # output DMAs: halves let the accum chain start early
                    for q2 in range(2):
                        nc.sync.dma_start(
                            out=xt_all[:, q2 * 4096:(q2 + 1) * 4096],
                            in_=xB[blk * 128:(blk + 1) * 128,
                                   q2 * 4096:(q2 + 1) * 4096])
                else:
                    nc.sync.dma_start(
                        out=xt_all, in_=xB[blk * 128:(blk + 1) * 128, :])
                xts = [xt_all[:, ht * 512:(ht + 1) * 512] for ht in range(16)]
                # q0..q3, k projections (feature-major out)
                for dt in range(QHL + 1):
                    ps = pacc.tile([128, 512], f32, tag="acc")
                    for ht in range(16):
                        nc.tensor.matmul(
                            ps,
                            lhsT=wqkv_sb[:, dt * 2048 + ht * 128:dt * 2048 + (ht + 1) * 128],
                            rhs=xts[ht], start=(ht == 0), stop=(ht == 15))
                    sq = tmp.tile([128, 512], bf16, tag="sq")
                    nc.scalar.activation(out=sq, in_=ps, func=ACT_F.Square)
                    traw = tmp.tile([128, 512], bf16, tag="traw")
                    nc.scalar.activation(out=traw, in_=ps, func=ACT_F.Copy)
                    ssq = pmm.tile([128, 512], f32, tag="mm")
                    nc.tensor.matmul(ssq, lhsT=ones_sb, rhs=sq, start=True, stop=True)
                    # rstd = exp(-0.5*ln(ssq/HD)) = 1/sqrt(ssq/HD) on ScalarE
                    lssq = tmp.tile([128, 512], f32, tag="std")
                    nc.scalar.activation(
                        out=lssq, in_=ssq, func=ACT_F.Ln, scale=1.0 / HD)
                    rstd = tmp.tile([128, 512], bf16, tag="rstd")
                    nc.scalar.activation(
                        out=rstd, in_=lssq, func=ACT_F.Exp, scale=-0.5)
                    cos_t, rot_t = (cq_sb, rq_sb) if dt < QHL else (ck_sb, rk_sb)
                    t1 = tmp.tile([128, 512], bf16, tag="t1")
                    nc.vector.tensor_mul(t1, traw, cos_t[:, t0:t0 + 512])
                    rps = pmm.tile([128, 512], f32, tag="mm")
                    nc.tensor.matmul(rps, lhsT=rot_t, rhs=traw, start=True, stop=True)
                    t2 = tmp.tile([128, 512], bf16, tag="t2")
                    nc.vector.tensor_mul(t2, rps, sin_sb[:, t0:t0 + 512])
                    nc.vector.tensor_add(out=t1, in0=t1, in1=t2)
                    dest = (qT[:, dt * T + t0:dt * T + t0 + 512] if dt < QHL
                            else kT[:, t0:t0 + 512])
                    nc.vector.tensor_mul(dest, t1, rstd)
                # v projection, natural layout [t_part, d_free]
                vps = pacc.tile([128, 512], f32, tag="acc")
                for c4 in range(4):
                    for ht in range(16):
                        nc.tensor.matmul(
                            vps[:, c4 * 128:(c4 + 1) * 128],
                            lhsT=xts[ht][:, c4 * 128:(c4 + 1) * 128],
                            rhs=wqkv_sb[:, 5 * 2048 + ht * 128:5 * 2048 + (ht + 1) * 128],
                            start=(ht == 0), stop=(ht == 15))
                with nc.allow_low_precision(reason="bf16 act copy"):
                    nc.vector.tensor_copy(
                        out=vn[:, blk * 512:(blk + 1) * 512], in_=vps)

            def attn_block(h, j):
                # Software-pipelined: S^T matmuls issued DEPTH tiles ahead so
                # the PE never stalls on the ACT exp of the current tile.
                # Causal restriction: tile i >= 4j only covers query columns
                # >= 128*(i-4j); the 128-wide boundary microblock gets the
                # shared upper-tri mask.
                DEPTH = cfg["depth"]
                ntk = 4 * j + 4
                aps = pacc.tile([128, 512], f32, tag="acc")
                dps = pden.tile([128, 512], f32, tag="den")
                sps_l, pt_l, c0_l = [], [], []

                def issue_st(i):
                    r = i - 4 * j
                    c0 = 128 * r if r > 0 else 0
                    sps = pmm.tile([128, 512], f32, tag="mm", name="sps")
                    nc.tensor.matmul(
                        sps[:, c0:], lhsT=kT[:, i * 128:(i + 1) * 128],
                        rhs=qT[:, h * T + j * 512 + c0:h * T + (j + 1) * 512],
                        start=True, stop=True)
                    sps_l.append(sps)
                    c0_l.append(c0)

                def issue_exp(i):
                    r = i - 4 * j
                    c0 = c0_l[i]
                    pt = tmp.tile([128, 512], bf16, tag="pt", name="pt")
                    nc.scalar.activation(
                        out=pt[:, c0:], in_=sps_l[i][:, c0:],
                        func=ACT_F.Exp, scale=SCALE)
                    if r >= 0:
                        # GpSimd (idle): keeps the microblock mask from
                        # queueing behind 3.3us DVE reciprocals
                        nc.gpsimd.tensor_mul(
                            pt[:, c0:c0 + 128], pt[:, c0:c0 + 128], tri_sb)
                    pt_l.append(pt)

                for i in range(min(DEPTH, ntk)):
                    issue_st(i)
                issue_exp(0)
                for i in range(ntk):
                    if i + DEPTH < ntk:
                        issue_st(i + DEPTH)
                    if i + 1 < ntk:
                        issue_exp(i + 1)
                    c0 = c0_l[i]
                    nc.tensor.matmul(dps[:, c0:], lhsT=ones_sb, rhs=pt_l[i][:, c0:],
                                     start=(i == 0), stop=(i == ntk - 1))
                    nc.tensor.matmul(aps[:, c0:], lhsT=vn[:, i * 128:(i + 1) * 128],
                                     rhs=pt_l[i][:, c0:], start=(i == 0),
                                     stop=(i == ntk - 1))
                recip = tmp.tile([128, 512], mybir.dt.float32, tag="rec")
                if h == QHL - 1:
                    # last head gates the next o_proj group: use the shorter
                    # ACT Ln->Exp chain instead of the 3.3us DVE reciprocal
                    lr = tmp.tile([128, 512], mybir.dt.float32, tag="lrec")
                    nc.scalar.activation(out=lr, in_=dps, func=ACT_F.Ln)
                    nc.scalar.activation(out=recip, in_=lr, func=ACT_F.Exp,
                                         scale=-1.0)
                else:
                    nc.vector.reciprocal(out=recip, in_=dps)
                nc.vector.tensor_mul(
                    attnT[:, h * T + j * 512:h * T + (j + 1) * 512], aps, recip)

            def oproj_row(m):
                # full 2048-wide output row of 128 tokens: 4 accumulations into
                # separate PSUM tiles, copies gathered into one SBUF tile, ONE
                # output DMA (512KB, 4KB/partition contiguous).
                osb = tmp.tile([128, 2048], bf16, tag="osb", name="osb")
                for j in range(NBLK):
                    ops = pmm.tile([128, 512], f32, tag="mm", name="ops")
                    for hh in range(QHL):
                        nc.tensor.matmul(
                            ops,
                            lhsT=attnT[:, hh * T + m * 128:hh * T + (m + 1) * 128],
                            rhs=wo_sb[:, hh * H + j * 512:hh * H + (j + 1) * 512],
                            start=(hh == 0), stop=(hh == QHL - 1))
                    if m >= 8 or (m + j) % 4 < 3:
                        with nc.allow_low_precision(reason="bf16 partials, host-summed f32"):
                            nc.vector.tensor_copy(
                                out=osb[:, j * 512:(j + 1) * 512], in_=ops)
                    else:
                        nc.scalar.activation(
                            out=osb[:, j * 512:(j + 1) * 512], in_=ops,
                            func=ACT_F.Copy)
                nc.sync.dma_start(
                    out=out[m * 128:(m + 1) * 128, :], in_=osb)

            # phase interleave: attention for query block j only needs
            # k/v/q blocks <= j; o_proj rows 4j..4j+3 only need attnT of
            # block j (all heads). Spreads ACT/DVE/DMA phases into the
            # PE-dense projection phase.
            phase1_block(0)
            # o_proj weights + mask load deferred past the cold-start
            # transfers, but early enough to beat the first o_proj row
            nc.scalar.dma_start(out=constsB_sb, in_=constsB[:, :])
            phase1_block(1)
            for h in range(QHL):
                attn_block(h, 0)
            phase1_block(2)
            for m in range(0, 4):
                oproj_row(m)
            for h in range(QHL):
                attn_block(h, 1)
            phase1_block(3)
            for m in range(4, 8):
                oproj_row(m)
            for h in range(QHL):
                attn_block(h, 2)
            for m in range(8, 12):
                oproj_row(m)
            for h in range(QHL):
                attn_block(h, 3)
            for m in range(12, 16):
                oproj_row(m)
    return nc


_GRAPH = None


def kernel(x, Wq, Wk, Wv, Wo, q_norm_w, k_norm_w):
    global _GRAPH, LAST_RESULTS
    x = np.asarray(x, dtype=np.float32)
    Wq = np.asarray(Wq, dtype=np.float32)
    Wk = np.asarray(Wk, dtype=np.float32)
    Wv = np.asarray(Wv, dtype=np.float32)
    Wo = np.asarray(Wo, dtype=np.float32)
    q_norm_w = np.asarray(q_norm_w, dtype=np.float32)
    k_norm_w = np.asarray(k_norm_w, dtype=np.float32)

    xT = np.ascontiguousarray(x.reshape(BT, H).T).astype(BF16)
    # pre-tiled blocks: xBm[bi*128+p, ht*512+c] = xT[ht*128+p, bi*512+c]
    xBm = np.ascontiguousarray(
        xT.reshape(16, 128, B * NBLK, 512).transpose(2, 1, 0, 3)
        .reshape(B * NBLK * 128, 16 * 512))
    cos_q, cos_k, sin_d, rotm_q, rotm_k = _rope_tables(q_norm_w, k_norm_w)
    p = np.arange(128)[:, None]
    f = np.arange(128)[None, :]
    tri = (f >= p).astype(BF16)       # upper-tri incl diagonal

    in_maps = []
    for c in range(NCORES):
        b, g = c // GPB, c % GPB
        w_all = np.concatenate([
            Wq[QHL * HD * g:QHL * HD * (g + 1)],
            Wk[HD * g:HD * (g + 1)],
            Wv[HD * g:HD * (g + 1)]], 0)              # [768, H]
        wqkvT = np.ascontiguousarray(w_all.T).astype(BF16)       # [H, 768]
        woT = np.ascontiguousarray(
            Wo[:, QHL * HD * g:QHL * HD * (g + 1)].T).astype(BF16)  # [QHL*HD, H]
        # dt-major packing: col = dt*2048 + ht*128 + q, so each 512KB
        # weight sub-DMA delivers one full feature tile in consumption order
        cw = np.ascontiguousarray(
            wqkvT.reshape(16, 128, NFT, 128).transpose(1, 2, 0, 3)
            .reshape(128, NCW)).astype(BF16)
        ca = np.zeros((128, NCA), dtype=BF16)
        ca[:, OFF_CQ:OFF_CQ + T] = cos_q
        ca[:, OFF_CK:OFF_CK + T] = cos_k
        ca[:, OFF_SIN:OFF_SIN + T] = sin_d
        ca[:, OFF_RQ:OFF_RQ + HD] = rotm_q
        ca[:, OFF_RK:OFF_RK + HD] = rotm_k
        ca[:, OFF_ONES:OFF_ONES + 128] = 1.0
        cb = np.zeros((128, NCB), dtype=BF16)
        cb[:, OFF_WO:OFF_WO + QHL * H] = (
            woT.reshape(QHL, 128, H).transpose(1, 0, 2).reshape(128, QHL * H))
        cb[:, OFF_TRI:OFF_TRI + 128] = tri
        in_maps.append({
            "xB": xBm[b * NBLK * 128:(b + 1) * NBLK * 128],
            "constsW": cw, "constsA": ca, "constsB": cb})

    if _GRAPH is None:
        import json as _json
        cfg = _json.loads(os.environ.get("ATTN_CFG", "{}")) or None
        _GRAPH = _legalize_waits(_build_graph(cfg=cfg))

    want_trace = bool(int(os.environ.get("ATTN_TRACE", "0")))
    try:
        res = run_bass_kernel_spmd(
            _GRAPH, in_maps, core_ids=list(range(NCORES)), trace=want_trace)
    except ModuleNotFoundError:
        if not want_trace:
            raise
        # axon NTFF profile hook unavailable in this environment
        res = run_bass_kernel_spmd(
            _GRAPH, in_maps, core_ids=list(range(NCORES)), trace=False)
    LAST_RESULTS = res
    acc = np.zeros((BT, H), dtype=np.float32)
    for c, r in enumerate(res.results):
        b = c // GPB
        acc[b * T:(b + 1) * T] += r["out"]
    return acc.reshape(B, T, H)
